# revision 16
# baseline (speedup 1.0000x reference)
"""Trainium2 Bass kernel for nn_EventMemoryCell (B=4096, D=H=512, S=16).

Strategy (hardcoded for the spec shapes):
  - Data parallel over batch across 8 cores (512 rows each), parameters
    replicated; one SPMD NEFF.
  - Everything on-device runs in a transposed (feature-on-partition,
    batch-on-free) layout, so every matmul contracts over partitions and
    the LSTM recurrence needs no transposes.
  - mem_seq is never materialized. Per step s the gate pre-activations
    are assembled in PSUM as
        64*(d (x) delta_s  +  A8@(slots_s + leak*x)  +  C8@cum_s
            + R  +  Wh8@h_s),
    activated with scale=1/64 and the exact f32 LSTM bias in the
    activation's per-partition bias operand.
      * slot tiles arrive f16 and are folded with leak*x (f16) into fp8
        on the otherwise-idle GPSIMD engine - a single quantization, so
        the constant leak*x carries no coherent rounding bias.
      * the delta column is a rank-1 fp8 DoubleRow matmul (K=2) that
        also opens each PSUM group.
      * R = 64*((A - A8)@(leak*x) + C@2x) collects every x-term whose
        error would otherwise repeat coherently across all 16 steps
        (fp8 weight residual on the A side; C@2x computed once with f16
        weights).  R is stored as an interleaved fp8 (hi, lo) pair and
        injected per tile with ONE DoubleRow matmul against (eye, eye)
        weights, giving ~f16 fidelity at fp8-DR cost.  This also absorbs
        the s=15 cum term (cum'[15] = 2x), so the last step needs no C
        matmul and no 16th cum tile.
  - Attention sims rows are computed from the raw f16 slot tiles
    (GPSIMD multiply one step ahead + ones-matmul partition reduction).
  - The big per-step GEMMs run in fp8-e4m3 DoubleRow (2 k-rows per PE
    cell), weights pre-scaled x64 on the host so 0.02-scale values
    clear the e4m3 denormal range.
  - Gate-tile emission is software-pipelined (PIPE groups of stream-side
    matmuls run ahead of the W_hh terms); the c/h chain for a finished
    chunk is deferred two gate activations so tanh(c) never blocks the
    in-order activation engine.  Stream DMAs run two steps ahead, cum
    tiles on the vector queue so the sync queue never backs up.
"""
import sys

sys.path.insert(0, "/opt/trn_rl_repo")

import numpy as np
import ml_dtypes

import concourse.bass as bass
import concourse.tile as tile
import concourse.mybir as mybir
from concourse import bacc, bass_utils

F32 = mybir.dt.float32
F32R = mybir.dt.float32r
F16 = mybir.dt.float16
FP8 = mybir.dt.float8e4
AF = mybir.ActivationFunctionType
DR = mybir.MatmulPerfMode.DoubleRow
ALU = mybir.AluOpType
NPF16 = np.float16
NPF8 = ml_dtypes.float8_e4m3fn

B, S, D, H = 4096, 16, 512, 512
NCORES = 8
BC = B // NCORES            # 512 batch rows per core
G4 = 4 * H                  # 2048 gate rows
KD = D // 128               # 4 k-tiles over D/H
KM = G4 // 128              # 16 gate partition tiles
KO = (2 * H + D) // 128     # 12 k-tiles for the output projection
WSCALE = 64.0               # fp8 weight prescale (undone in gate activation)

# interleaved gate-tile order [0,4,8,12, 1,5,9,13, ...]: finish chunk j's
# i/f/g/o gates together so c/h updates start early
M_ORDER = [j + 4 * i for j in range(4) for i in range(4)]
PIPE = 6                    # gate-tile groups opened ahead of their Wh terms
CHAIN_DEFER = 2             # gate acts emitted between chunk ready and chain

_BUILT = None
DEBUG_R = False
DEBUG_STEPS = ()  # set before first kernel() call to dump c/h after these steps


def _build_program():
    nc = bacc.Bacc("TRN2", target_bir_lowering=False, debug=False)

    def din(name, shape, dt):
        return nc.dram_tensor(name, list(shape), dt, kind="ExternalInput").ap()

    xT = din("xT", (D, BC), F16)
    s0T = din("s0T", (D, BC), F16)
    slotsT = din("slotsT", (S - 1, D, BC), F16)
    cumT = din("cumT", (S - 1, D, BC), FP8)      # raw cum_old[s+1]
    d8T = din("d8T", (1, 2, (S - 1) * BC), FP8)  # rows (delta+1, 0)
    hpT = din("hpT", (H, BC), F16)
    m2T = din("m2T", (D, D + 2), F16)  # cols D..D+1: wr, wa
    wvT = din("wvT", (D, D), F16)
    smalls = din("smalls", (D, 10), F32)         # wr|wa|bv|bo|bias2(4)|br|ba
    dd8 = din("dd8", (1, 2, G4), FP8)            # rows (64*d_col, 0)
    a8 = din("a8", (D, G4), FP8)                 # x64
    c8 = din("c8", (D, G4), FP8)                 # x64
    wh8 = din("wh8", (H, G4), FP8)               # x64
    ea64 = din("ea64", (D, G4), FP8)             # 64*(A - A8), transposed
    c64T = din("c64T", (D, G4), F16)             # 128*C, transposed
    eye2 = din("eye2", (128, 2, 128), FP8)       # rows (eye, eye)
    woT = din("woT", (2 * H + D, H), F16)
    hnT = nc.dram_tensor("hnT", [H, BC], F32, kind="ExternalOutput").ap()
    RD = (nc.dram_tensor("RD", [128, KM * 2 * BC], FP8, kind="ExternalOutput").ap()
          if DEBUG_R else None)
    dbg = {}
    for ds in DEBUG_STEPS:
        dbg[ds] = (nc.dram_tensor(f"cD{ds}", [H, BC], F32, kind="ExternalOutput").ap(),
                   nc.dram_tensor(f"hD{ds}", [H, BC], F32, kind="ExternalOutput").ap())

    r3 = lambda ap: ap.rearrange("(kt p) b -> p kt b", p=128)
    r2 = lambda ap: ap.rearrange("(kt p) o -> p (kt o)", p=128)

    with tile.TileContext(nc) as tc:
        wp = tc.alloc_tile_pool(name="wp", bufs=1)
        st_p = tc.alloc_tile_pool(name="state", bufs=1)
        pp = tc.alloc_tile_pool(name="pp", bufs=8, space="PSUM")

        # ---- resident weights / constants ----
        a8_sb = wp.tile([128, KD, G4], FP8, name="a8_sb")
        c8_sb = wp.tile([128, KD, G4], FP8, name="c8_sb")
        wh8_sb = wp.tile([128, KD, G4], FP8, name="wh8_sb")
        dd8_sb = wp.tile([1, 2, G4], FP8, name="dd8_sb")
        eye2_sb = wp.tile([128, 2, 128], FP8, name="eye2_sb")
        d8_sb = wp.tile([1, 2, (S - 1) * BC], FP8, name="d8_sb")
        sm_sb = wp.tile([128, KD, 10], F32, name="sm_sb")
        R_sb = wp.tile([128, KM, 2, BC], FP8, name="R_sb")
        wo_sb = wp.tile([128, KO, H], F16, name="wo_sb")
        ones_bf = wp.tile([1, BC], F16, name="ones_bf")
        nc.vector.memset(ones_bf[:], 1.0)
        ones8 = wp.tile([128, 2, 16], FP8, name="ones8")
        nc.vector.memset(ones8[:], 1.0)

        sp = tc.alloc_tile_pool(name="sp", bufs=3)   # raw f16 slot tiles
        fp_ = tc.alloc_tile_pool(name="fp", bufs=3)  # folded fp8 slot tiles
        cp = tc.alloc_tile_pool(name="cp", bufs=3)   # cum tiles
        tsp = tc.alloc_tile_pool(name="tsp", bufs=2)  # sims product tiles

        xt = st_p.tile([128, KD, BC], F16, name="xt")
        ut = st_p.tile([128, KD, BC], FP8, name="ut")
        P_t = st_p.tile([128, KD, BC], F16, name="P_t")
        lx16 = st_p.tile([128, KD, BC], F16, name="lx16")
        lx8 = st_p.tile([128, KD, BC], FP8, name="lx8")
        c_t = [st_p.tile([128, BC], F16, name=f"c{k}", tag=f"c{k}") for k in range(KD)]
        h8 = [st_p.tile([128, KD, BC], FP8, name=f"h8_{pq}", tag=f"h8_{pq}")
              for pq in range(2)]
        h15 = st_p.tile([128, KD, BC], F16, name="h15")
        g_row = st_p.tile([1, BC], F16, name="g_row")
        max_row = st_p.tile([1, BC], F32, name="max_row")
        hp_sb = st_p.tile([128, KD, BC], F16, name="hp_sb")
        ns8 = st_p.tile([128, KD, BC], FP8, name="ns8")

        MSIG, MTANH = AF.Sigmoid, AF.Tanh

        def mm_group(ps_ap, terms):
            n = len(terms)
            for i, term in enumerate(terms):
                pm = term[2] if len(term) > 2 else None
                nc.tensor.matmul(ps_ap, term[0], term[1], start=(i == 0),
                                 stop=(i == n - 1), perf_mode=pm)

        # stream-tile state, shared between prologue and the step loop
        st_t = [None] * S
        fs_t = [None] * S
        ct_t = [None] * S
        tsim_t = [None] * S

        def stream_dma(s):
            if s > S - 2:
                return
            ct = cp.tile([128, KD, BC], FP8, name=f"ct{s}", tag="ct")
            nc.scalar.dma_start(ct[:], r3(cumT[s]))
            ct_t[s] = ct
            st = sp.tile([128, KD, BC], F16, name=f"st{s}", tag="st")
            nc.sync.dma_start(st[:], r3(slotsT[s]))
            st_t[s] = st

        def emit_fold(s):
            if s > S - 2:
                return
            fs = fp_.tile([128, KD, BC], FP8, name=f"fs{s}", tag="fs")
            nc.gpsimd.tensor_add(fs[:], st_t[s][:], lx16[:])
            fs_t[s] = fs

        def emit_tsim_mul(s):
            # product tile for sims row s+1, from the raw f16 slot tile
            tsim = tsp.tile([128, KD, BC], FP8, name=f"tm{s}", tag="tsim")
            nc.gpsimd.tensor_mul(tsim[:], st_t[s][:], ut[:])
            tsim_t[s] = tsim

        def sims_reduce(idx, tsrc):
            # running max over slots: reduce tsrc over partitions via ones
            srp = pp.tile([128, BC], F32, name=f"srp{idx}", tag="ps")
            mm_group(srp[0:1, :], [(ones8[:, :, 0:1], tsrc[:, 2 * t:2 * t + 2, :], DR)
                                   for t in (0, 1)])
            if idx == 0:
                nc.scalar.activation(max_row[:], srp[0:1, :], AF.Copy)
            else:
                nc.vector.tensor_max(max_row[:], max_row[:], srp[0:1, :])

        # ================= prologue =================
        with tc.tile_pool(name="prop", bufs=1) as prop:
            # scalar-queue bulk loads, issued first so they stream in
            # parallel with the sync/vector-queue loads below
            wv_sb = prop.tile([128, KD, D], F16, name="wv_sb")
            nc.scalar.dma_start(wv_sb[:], r3(wvT))
            nc.scalar.dma_start(a8_sb[:], r3(a8))
            nc.scalar.dma_start(c8_sb[:], r3(c8))
            nc.scalar.dma_start(wh8_sb[:], r3(wh8))
            nc.scalar.dma_start(hp_sb[:], r3(hpT))
            ea_sb = prop.tile([128, KD, G4], FP8, name="ea_sb")
            nc.scalar.dma_start(ea_sb[:], r3(ea64))
            c64_sb = prop.tile([128, KD, G4], F16, name="c64_sb")
            for half in (0, 1):
                cs = slice(1024 * half, 1024 * (half + 1))
                nc.scalar.dma_start(c64_sb[:, :, cs], r3(c64T)[:, :, cs])
            # sync-queue critical path
            nc.sync.dma_start(xt[:], r3(xT))
            m2_sb = prop.tile([128, KD, D + 2], F16, name="m2_sb")
            nc.sync.dma_start(m2_sb[:], r3(m2T))
            nc.sync.dma_start(sm_sb[:], r3(smalls))
            s0_sb = prop.tile([128, KD, BC], F16, name="s0_sb")
            nc.sync.dma_start(s0_sb[:], r3(s0T))
            nc.sync.dma_start(d8_sb[:], d8T)
            nc.sync.dma_start(dd8_sb[:], dd8)
            nc.sync.dma_start(eye2_sb[:], eye2)
            vt = prop.tile([128, KD, BC], F16, name="vt")
            r_row = prop.tile([1, BC], F16, name="r_row")
            lk_row = prop.tile([1, BC], F16, name="lk_row")
            R_bc = prop.tile([128, BC], F16, name="R_bc")
            L_bc = prop.tile([128, BC], F16, name="L_bc")


            # u = (Wk^T Wq) x ; v = Wv x + bv   (feature-major)
            for m in range(KD):
                ups = pp.tile([128, BC], F32, name=f"ups{m}", tag="ps")
                mm_group(ups[:], [(m2_sb[:, k, 128 * m:128 * (m + 1)], xt[:, k, :])
                                  for k in range(KD)])
                nc.scalar.activation(ut[:, m, :], ups[:], AF.Copy)
            for m in range(KD):
                vps = pp.tile([128, BC], F32, name=f"vps{m}", tag="ps")
                mm_group(vps[:], [(wv_sb[:, k, 128 * m:128 * (m + 1)], xt[:, k, :])
                                  for k in range(KD)])
                nc.scalar.activation(vt[:, m, :], vps[:], AF.Identity,
                                     bias=sm_sb[:, m, 2:3])

            # r / leak rows
            rps = pp.tile([128, BC], F32, name="rps", tag="ps")
            mm_group(rps[0:1, :], [(m2_sb[:, k, D:D + 1], xt[:, k, :]) for k in range(KD)])
            nc.scalar.activation(r_row[:], rps[0:1, :], MSIG, bias=sm_sb[0:1, 0, 8:9])
            lps = pp.tile([128, BC], F32, name="lps", tag="ps")
            mm_group(lps[0:1, :], [(m2_sb[:, k, D + 1:D + 2], xt[:, k, :]) for k in range(KD)])
            nc.scalar.activation(lk_row[:], lps[0:1, :], MSIG, bias=sm_sb[0:1, 0, 9:10])

            # broadcast r/leak rows to 128 partitions via a K=1 matmul
            bps = pp.tile([128, BC], F32, name="bps", tag="ps")
            mm_group(bps[:], [(ones_bf[0:1, 0:128], r_row[:])])
            nc.scalar.activation(R_bc[:], bps[:], AF.Copy)
            bps2 = pp.tile([128, BC], F32, name="bps2", tag="ps")
            mm_group(bps2[:], [(ones_bf[0:1, 0:128], lk_row[:])])
            nc.scalar.activation(L_bc[:], bps2[:], AF.Copy)

            # P = r*slots0 + (1-r)*v = v + r*(slots0 - v);  lx = leak*x
            for k in range(KD):
                t1 = prop.tile([128, BC], F16, name=f"pt{k}", tag="ptmp", bufs=2)
                nc.vector.tensor_sub(t1[:], s0_sb[:, k, :], vt[:, k, :])
                nc.vector.tensor_mul(t1[:], R_bc[:], t1[:])
                nc.vector.tensor_add(P_t[:, k, :], vt[:, k, :], t1[:])
                nc.vector.tensor_mul(lx16[:, k, :], L_bc[:], xt[:, k, :])
                nc.vector.tensor_mul(lx8[:, k, :], L_bc[:], xt[:, k, :])

            # R = 64*((A-A8)@lx + C@2x), stored fp8 (hi, lo) interleaved
            for m in range(KM):
                sl = slice(128 * m, 128 * (m + 1))
                rps_ = pp.tile([128, BC], F32, name=f"rfx{m}", tag="ps")
                terms = [(ea_sb[:, 2 * t:2 * t + 2, sl],
                          lx8[:, 2 * t:2 * t + 2, :], DR) for t in (0, 1)]
                terms += [(c64_sb[:, k, sl], xt[:, k, :]) for k in range(KD)]
                mm_group(rps_[:], terms)
                # stored at quarter scale so fp8 never overflows; the eye
                # injection weights carry the 4x back
                nc.scalar.activation(R_sb[:, m, 0, :], rps_[:], AF.Copy,
                                     scale=0.25)
                # lo = psum/4 - hi, fp8
                nc.vector.scalar_tensor_tensor(
                    R_sb[:, m, 1, :], rps_[:], 0.25, R_sb[:, m, 0, :],
                    ALU.mult, ALU.subtract)

            if DEBUG_R:
                nc.sync.dma_start(RD.rearrange("p (m two b) -> p m two b", m=KM, two=2, b=BC), R_sb[:])
            # sims row 0 (original slot 0)
            ts0 = prop.tile([128, KD, BC], FP8, name="ts0")
            nc.vector.tensor_mul(ts0[:], s0_sb[:], ut[:])
            sims_reduce(0, ts0)

            stream_dma(0)
            stream_dma(1)
            emit_fold(0)
            emit_tsim_mul(0)
            emit_fold(1)

        # ================= LSTM over S steps =================
        gp = tc.alloc_tile_pool(name="gp", bufs=10)
        tp = tc.alloc_tile_pool(name="tp", bufs=2)
        gates_t = {}
        act_count = [0]       # global gate-act counter (for chain deferral)
        chain_q = []          # (ready_at_count, s, j)

        def emit_chain(s, j):
            last = s == S - 1
            gates = gates_t[s]
            h_wr = h15 if last else h8[s % 2]
            ig, fg, gg, og = (gates[j], gates[4 + j], gates[8 + j],
                              gates[12 + j])
            tct = tp.tile([128, BC], F16, name=f"t_{s}_{j}", tag="tct")
            if s == 0:
                nc.vector.tensor_mul(c_t[j][:], ig[:], gg[:])
            else:
                t2 = tp.tile([128, BC], F16, name=f"u_{s}_{j}", tag="t2")
                nc.vector.tensor_mul(t2[:], fg[:], c_t[j][:])
                nc.vector.tensor_mul(c_t[j][:], ig[:], gg[:])
                nc.vector.tensor_add(c_t[j][:], c_t[j][:], t2[:])
            nc.scalar.activation(tct[:], c_t[j][:], MTANH)
            nc.vector.tensor_mul(h_wr[:, j, :], og[:], tct[:])
            if j == KD - 1 and s in dbg:
                cD, hD = dbg[s]
                for jj in range(KD):
                    ccp = tp.tile([128, BC], F32, name=f"ccp{s}_{jj}",
                                  tag="ccp")
                    nc.scalar.activation(ccp[:], c_t[jj][:], AF.Copy)
                    nc.sync.dma_start(cD[128 * jj:128 * (jj + 1), :], ccp[:])
                    hcp = tp.tile([128, BC], F32, name=f"hcp{s}_{jj}",
                                  tag="hcp")
                    nc.scalar.activation(hcp[:], h_wr[:, jj, :], AF.Copy)
                    nc.sync.dma_start(hD[128 * jj:128 * (jj + 1), :],
                                      hcp[:])

        def flush_chains(force=False):
            while chain_q and (force or chain_q[0][0] <= act_count[0]):
                _, s_, j_ = chain_q.pop(0)
                emit_chain(s_, j_)

        def emit_ns():
            # g = sigmoid(max_s sims);  ns8 = g*P + leak*x
            nc.scalar.activation(g_row[:], max_row[:], MSIG)
            gps = pp.tile([128, BC], F32, name="gps", tag="ps")
            mm_group(gps[:], [(ones_bf[0:1, 0:128], g_row[:])])
            G_bc = tp.tile([128, BC], F16, name="G_bc", tag="gbc", bufs=1)
            nc.scalar.activation(G_bc[:], gps[:], AF.Copy)
            for k in range(KD):
                t2 = tp.tile([128, BC], F16, name=f"nsm{k}", tag="nsm", bufs=2)
                nc.vector.tensor_mul(t2[:], G_bc[:], P_t[:, k, :])
                nc.vector.tensor_add(ns8[:, k, :], t2[:], lx16[:, k, :])

        def emit_pre(s, m):
            last = s == S - 1
            sl = slice(128 * m, 128 * (m + 1))
            ps = pp.tile([128, BC], F32, name=f"ps_{s}_{m}", tag="ps")
            pre = []
            if not last:
                # rank-1 delta term opens the group (K=2 fp8 DoubleRow)
                pre.append((dd8_sb[:, :, sl],
                            d8_sb[:, :, s * BC:(s + 1) * BC], DR))
            src = ns8 if last else fs_t[s]
            for t in (0, 1):
                pre.append((a8_sb[:, 2 * t:2 * t + 2, sl],
                            src[:, 2 * t:2 * t + 2, :], DR))
            if not last:
                for t in (0, 1):
                    pre.append((c8_sb[:, 2 * t:2 * t + 2, sl],
                                ct_t[s][:, 2 * t:2 * t + 2, :], DR))
            # constant x-terms: hi/lo pair via one DoubleRow eye matmul
            pre.append((eye2_sb[:], R_sb[:, m, :, :], DR))
            fin = []
            if s > 0:
                for t in (0, 1):
                    fin.append((wh8_sb[:, 2 * t:2 * t + 2, sl],
                                h8[(s + 1) % 2][:, 2 * t:2 * t + 2, :], DR))
            n_all = len(pre) + len(fin)
            for i, term in enumerate(pre):
                nc.tensor.matmul(ps[:], term[0], term[1],
                                 start=(i == 0),
                                 stop=(i == n_all - 1),
                                 perf_mode=term[2])
            return ps, fin, len(pre)

        def emit_fin(s, m, ps, fin, npre):
            n_all = npre + len(fin)
            for i, term in enumerate(fin):
                nc.tensor.matmul(ps[:], term[0], term[1], start=False,
                                 stop=(npre + i == n_all - 1),
                                 perf_mode=term[2])
            gt = gp.tile([128, BC], F16, name=f"g_{s}_{m}", tag="gate")
            nc.scalar.activation(gt[:], ps[:],
                                 MTANH if m // 4 == 2 else MSIG,
                                 scale=1.0 / WSCALE,
                                 bias=sm_sb[:, m % 4, 4 + m // 4:5 + m // 4])
            gates_t[s][m] = gt
            act_count[0] += 1
            j = m - 12
            if j >= 0:
                chain_q.append((act_count[0] + CHAIN_DEFER, s, j))
            flush_chains()

        for s in range(S):
            # deferred chains of step s-1 must be emitted before any step-s
            # Wh matmul reads h8, or those reads would bind to stale writes
            flush_chains(force=True)
            gates_t[s] = [None] * KM
            pend = []
            if s == 10:
                nc.scalar.dma_start(wo_sb[:], r3(woT))
            for idx, m in enumerate(M_ORDER):
                if idx == 2:
                    stream_dma(s + 2)
                if idx == 3 and s <= S - 2:
                    sims_reduce(s + 1, tsim_t[s])
                if idx == 4 and s == S - 2:
                    emit_ns()
                if idx == 5 and s + 1 <= S - 2:
                    emit_tsim_mul(s + 1)
                if idx == 7:
                    emit_fold(s + 2)
                pend.append((s, m, *emit_pre(s, m)))
                if len(pend) > PIPE:
                    emit_fin(*pend.pop(0))
            # drain before the next step's pres: the final chunks' c/h
            # chains then overlap the next step's lead-in matmuls
            while pend:
                emit_fin(*pend.pop(0))
        flush_chains(force=True)

        # ================= epilogue =================
        # (hp_sb/x16 were loaded back in the prologue; h15 terms go last in
        # each group so the matmuls start before the final h chain resolves)
        with tc.tile_pool(name="ep", bufs=1) as ep:
            eps_l = []
            for m in range(KD):
                sl = slice(128 * m, 128 * (m + 1))
                eps = pp.tile([128, BC], F32, name=f"eps{m}", tag="ps")
                terms = [(wo_sb[:, 4 + j, sl], hp_sb[:, j, :]) for j in range(KD)]
                terms += [(wo_sb[:, 8 + j, sl], xt[:, j, :]) for j in range(KD)]
                for i, term in enumerate(terms):
                    nc.tensor.matmul(eps[:], term[0], term[1], start=(i == 0),
                                     stop=False)
                eps_l.append(eps)
            # h15 terms interleaved by chunk j: terms for chunk j run as soon
            # as that chunk's c/h chain finishes; only j=3 sits on the tail
            for j in range(KD):
                for m in range(KD):
                    sl = slice(128 * m, 128 * (m + 1))
                    nc.tensor.matmul(eps_l[m][:], wo_sb[:, j, sl], h15[:, j, :],
                                     start=False, stop=(j == KD - 1))
            for m in range(KD):
                out_t = ep.tile([128, BC], F32, name=f"o{m}", tag="out", bufs=2)
                nc.scalar.activation(out_t[:], eps_l[m][:], MTANH,
                                     bias=sm_sb[:, m, 3:4])
                nc.sync.dma_start(hnT[128 * m:128 * (m + 1), 0:256],
                                  out_t[:, 0:256])
                nc.sync.dma_start(hnT[128 * m:128 * (m + 1), 256:512],
                                  out_t[:, 256:512])

        tp.release()
        gp.release()
        tsp.release()
        cp.release()
        fp_.release()
        sp.release()
        pp.release()
        st_p.release()
        wp.release()

    nc.compile()
    return nc


def kernel(**inputs):
    global _BUILT
    if _BUILT is None:
        _BUILT = _build_program()
    nc = _BUILT

    f32 = np.float32
    x = np.asarray(inputs["x_t"], f32)
    hp = np.asarray(inputs["h_prev"], f32)
    slots = np.asarray(inputs["slots"], f32)
    cum = np.asarray(inputs["cum_feats"], f32)
    dt = np.asarray(inputs["delta_t"], f32)
    Wk = np.asarray(inputs["Wk"], f32)
    Wq = np.asarray(inputs["Wq"], f32)
    Wv = np.asarray(inputs["Wv"], f32)
    bv = np.asarray(inputs["bv"], f32)
    Wr = np.asarray(inputs["Wr"], f32)
    br = np.asarray(inputs["br"], f32)
    Wa = np.asarray(inputs["Wa"], f32)
    ba = np.asarray(inputs["ba"], f32)
    W_ih = np.asarray(inputs["W_ih"], f32)
    W_hh = np.asarray(inputs["W_hh"], f32)
    b_ih = np.asarray(inputs["b_ih"], f32)
    b_hh = np.asarray(inputs["b_hh"], f32)
    Wo = np.asarray(inputs["Wo"], f32)
    bo = np.asarray(inputs["bo"], f32)

    xT = np.ascontiguousarray(x.T).astype(NPF16)
    hpT = hp.T.astype(NPF16)
    s0T = slots[:, 0, :].T.astype(NPF16)
    slotsT = slots[:, 1:, :].transpose(1, 2, 0).astype(NPF16)
    cumT = cum[:, 1:, :].transpose(1, 2, 0).astype(NPF8)
    d1T = np.ascontiguousarray((dt[:, 1:] + 1.0).T)  # (S-1, B)

    m2T = np.concatenate([Wq.T @ Wk, Wr.reshape(D, 1),
                          Wa.reshape(D, 1)], 1).astype(NPF16)
    wvT = np.ascontiguousarray(Wv.T).astype(NPF16)
    bias2 = (b_ih + b_hh).astype(f32)
    smalls = np.zeros((D, 10), f32)
    smalls[:, 0] = Wr.reshape(D)
    smalls[:, 1] = Wa.reshape(D)
    smalls[:, 2] = bv
    smalls[:, 3] = bo
    for cq in range(4):
        smalls[:, 4 + cq] = bias2.reshape(16, 128)[cq * 4:(cq + 1) * 4].reshape(D)
    smalls[0, 8] = br.reshape(())
    smalls[0, 9] = ba.reshape(())
    A = W_ih[:, :D]
    C = W_ih[:, D:2 * D]
    a8np = (WSCALE * A).astype(NPF8)
    c8np = (WSCALE * C).astype(NPF8)
    a8 = np.ascontiguousarray(a8np.T)
    c8 = np.ascontiguousarray(c8np.T)
    ea64 = np.ascontiguousarray(
        (WSCALE * A - a8np.astype(f32)).astype(NPF8).T)
    c64T = np.ascontiguousarray((2.0 * WSCALE * C).T).astype(NPF16)
    wh8 = np.ascontiguousarray((WSCALE * W_hh).T).astype(NPF8)
    dd8 = np.stack([WSCALE * W_ih[:, 2 * D],
                    np.zeros(G4, f32)]).reshape(1, 2, G4).astype(NPF8)
    eye = 4.0 * np.eye(128, dtype=f32)
    eye2 = np.stack([eye, eye], 1).reshape(128, 2, 128).astype(NPF8)
    woT = Wo.T.astype(NPF16)

    shared_w = {
        "m2T": m2T, "wvT": wvT, "smalls": smalls, "dd8": dd8,
        "a8": a8, "c8": c8, "wh8": wh8, "ea64": ea64, "c64T": c64T,
        "eye2": eye2, "woT": woT,
    }
    in_maps = []
    zrow = np.zeros((S - 1) * BC, f32)
    for c in range(NCORES):
        lo, hi = c * BC, (c + 1) * BC
        m = dict(shared_w)
        m["xT"] = xT[:, lo:hi]
        m["hpT"] = hpT[:, lo:hi]
        m["s0T"] = s0T[:, lo:hi]
        m["slotsT"] = slotsT[:, :, lo:hi]
        m["cumT"] = cumT[:, :, lo:hi]
        dsl = d1T[:, lo:hi].reshape((S - 1) * BC)
        m["d8T"] = np.stack([dsl, zrow]).reshape(1, 2, (S - 1) * BC).astype(NPF8)
        in_maps.append(m)

    res = bass_utils.run_bass_kernel_spmd(nc, in_maps, core_ids=list(range(NCORES)),
                                          **_RUN_KWARGS)
    global _LAST_RESULTS
    _LAST_RESULTS = res

    out = np.empty((B, H), np.float32)
    for c in range(NCORES):
        out[c * BC:(c + 1) * BC, :] = res.results[c]["hnT"].T
    return out


_RUN_KWARGS = {}
_LAST_RESULTS = None
